# revision 1
# baseline (speedup 1.0000x reference)
"""BoltzmannRouter Trainium2 kernel: 8-core data-parallel Bass implementation.

Full inputs: x (4, 4096, 2048) f32, gate_w (64, 2048) f32.
Output: routing weights (4, 4096, 64) f32 (softmax -> top-44 mask -> renorm).

Sharding: 16384 tokens split 2048/core across 8 NeuronCores; gate weight
replicated. Host pre-transposes each x shard to [D, tokens] so the device
DMA loads contraction-major tiles at full bandwidth, and pre-scales gate_w
by 1/TEMPERATURE (and 2^6 in the fp16 path).

Matmul precision modes (BOLTZ_MM_MODE):
  fp16x3 (default): x and w each split into fp16 high + 2^-12-scaled fp16 low
    parts; scores = 2^-6*(A + 2^-12*B) with A = xh@wh, B = xh@wl + xl@wh
    accumulated in separate PSUM banks. Dropped terms ~2^-22 relative --
    below fp32 PSUM accumulation noise -- at 3 cyc/row instead of fp32's 4.
  fp32: native fp32 matmul (2 half-rate passes per matmul).
"""

import os
import sys

sys.path.insert(0, "/opt/trn_rl_repo")

import numpy as np

D = 2048
E = 64
N_BOTTOM = 20  # 64 experts - 44 active
EPS = 1e-8
NEG_BIG = -1e30
TEMPERATURE = 2.718281828459045
N_CORES = 8
TPC = 2048  # tokens per core
GROUP = 512  # tokens per matmul group (one PSUM bank)

W_SCALE = 64.0  # 2^6: lifts gate_w into fp16-normal range
LO_SCALE = 4096.0  # 2^12: scale on the low fp16 split parts

_MODE = os.environ.get("BOLTZ_MM_MODE", "fp16x3")


def _build_nc():
    import concourse.bacc as bacc
    import concourse.mybir as mybir
    from concourse.masks import make_identity
    from concourse.tile import TileContext

    F32 = mybir.dt.float32
    F16 = mybir.dt.float16
    fp16 = _MODE == "fp16x3"
    mm_dt = F16 if fp16 else getattr(mybir.dt, _MODE, F32)
    kc_n = D // 128
    n_groups = TPC // GROUP
    n_sub = GROUP // 128
    # psum_t carries (-scores) scaled by W_SCALE in the fp16 path
    inv_s = 1.0 / W_SCALE if fp16 else 1.0

    lean_tail = os.environ.get("BOLTZ_LEAN_TAIL", "1") == "1"
    if lean_tail:
        # the stock Tile exit emits drain + barrier + sem-clear + barrier
        # (~8us); the kernel preamble already range-clears the semaphores at
        # the start of every execution, so drain + one barrier suffices
        def _lean_drain_and_barrier(self, tick_clock, wait_clock):
            from concourse.tile import ScopedClock

            drain_inst = self.nc.sync.drain()
            wait_clock.add_sem_waits(
                drain_inst.ins, ScopedClock({None: tick_clock.global_clock})
            )
            self.nc.all_engine_barrier()
            popped = self.nc._tile_sem_poison_stack.pop()
            assert popped is self._sem_poison
            self.sems.allocated()

        TileContext._drain_and_barrier = _lean_drain_and_barrier

    nc = bacc.Bacc(None, target_bir_lowering=False)
    if fp16:
        # xpk[d, g, 0, :] = xh tokens of group g, xpk[d, g, 1, :] = xl
        xpk_d = nc.declare_dram_parameter(
            "xpk", [D, (TPC // GROUP) * 2 * GROUP], F16, isOutput=False
        )
        whl_d = nc.declare_dram_parameter("whl", [D, 2 * E], F16, isOutput=False)
    else:
        xT = nc.declare_dram_parameter("xT", [D, TPC], mm_dt, isOutput=False)
        wT = nc.declare_dram_parameter("wT", [D, E], mm_dt, isOutput=False)
    out = nc.declare_dram_parameter("out", [TPC, E], F32, isOutput=True)

    with TileContext(nc) as tc:
        with (
            tc.tile_pool(name="const", bufs=1) as cpool,
            tc.tile_pool(name="xg", bufs=4) as xpool,
            tc.tile_pool(name="sneg", bufs=2) as spool,
            tc.tile_pool(name="og", bufs=4) as opool,
            tc.tile_pool(name="work", bufs=3) as wkpool,
            tc.tile_pool(name="small", bufs=8) as smpool,
            tc.tile_pool(name="ps_s", bufs=2 if fp16 else 2, space="PSUM") as ps_s_pool,
            tc.tile_pool(name="ps_b", bufs=2, space="PSUM") as ps_b_pool,
            tc.tile_pool(name="ps_t", bufs=4, space="PSUM") as ps_t_pool,
        ):
            ident = cpool.tile([E, E], F32)
            make_identity(nc, ident)
            if fp16:
                # -I/W_SCALE: transposing with a normal matmul by this matrix
                # descales and negates the scores in one shot
                identn = cpool.tile([E, E], F32)
                nc.gpsimd.memset(identn, 0.0)
                nc.gpsimd.affine_select(
                    out=identn,
                    in_=identn,
                    compare_op=mybir.AluOpType.not_equal,
                    fill=-1.0 / W_SCALE,
                    base=0,
                    pattern=[[-1, E]],
                    channel_multiplier=1,
                )

            if fp16:
                whl_sb = cpool.tile([128, kc_n, 2 * E], F16)
                nc.sync.dma_start(
                    out=whl_sb, in_=whl_d[:, :].rearrange("(kc p) e -> p kc e", p=128)
                )
            else:
                w_sb = cpool.tile([128, kc_n, E], mm_dt)
                nc.sync.dma_start(
                    out=w_sb, in_=wT[:, :].rearrange("(kc p) e -> p kc e", p=128)
                )

            og_tiles = []
            for g in range(n_groups):
                tok = slice(g * GROUP, (g + 1) * GROUP)
                if fp16:
                    # per-(group, chunk) tiles so PE pipelines at DMA-arrival
                    # granularity; dispatches spread over 3 queues (SP issue
                    # cost is ~620ns per DMA regardless of size)
                    xhs, xls = [], []
                    gcols = slice(g * 2 * GROUP, (g + 1) * 2 * GROUP)
                    for kc in range(kc_n):
                        row = slice(kc * 128, (kc + 1) * 128)
                        xk = xpool.tile([128, 2 * GROUP], F16, tag=f"x{kc}")
                        nc.sync.dma_start(out=xk, in_=xpk_d[row, gcols])
                        xhs.append(xk[:, :GROUP])
                        xls.append(xk[:, GROUP:])
                    # packed stationary [wh|wl]: one matmul against xh gives
                    # A=wh.T@xh (rows 0:64) and B1=wl.T@xh (rows 64:128); the
                    # second against xl gives B2=wh.T@xl (rows 0:64, the
                    # wl.T@xl block is a free byproduct, never read).
                    # The last parent group is processed in two half-width
                    # passes so the final selection chain drains sooner.
                    snegs = []
                    splits = (
                        [(0, GROUP)]
                        if g < n_groups - 1
                        else [(0, GROUP // 2), (GROUP // 2, GROUP // 2)]
                    )
                    for xoff, w in splits:
                        ps1 = ps_s_pool.tile([2 * E, w], F32, tag="ps_a")
                        ps2 = ps_b_pool.tile([2 * E, w], F32, tag="ps_b")
                        for kc in range(kc_n):
                            nc.tensor.matmul(
                                ps1,
                                lhsT=whl_sb[:, kc, :],
                                rhs=xhs[kc][:, xoff : xoff + w],
                                start=(kc == 0), stop=(kc == kc_n - 1),
                            )
                        for kc in range(kc_n):
                            nc.tensor.matmul(
                                ps2,
                                lhsT=whl_sb[:, kc, :],
                                rhs=xls[kc][:, xoff : xoff + w],
                                start=(kc == 0), stop=(kc == kc_n - 1),
                            )
                        # sneg = A + 2^-12 (B1 + B2) = W_SCALE * scores (sign
                        # and descale are folded into the transpose matrix)
                        b2_sb = spool.tile([E, w], F32, tag="b2_sb")
                        nc.scalar.copy(b2_sb, ps2[:E, :])
                        bs = spool.tile([E, w], F32, tag="bs")
                        nc.vector.tensor_add(bs, ps1[E:, :], b2_sb)
                        sneg = spool.tile([E, w], F32, tag="sneg")
                        nc.vector.scalar_tensor_tensor(
                            out=sneg,
                            in0=bs,
                            scalar=1.0 / LO_SCALE,
                            in1=ps1[:E, :],
                            op0=mybir.AluOpType.mult,
                            op1=mybir.AluOpType.add,
                        )
                        snegs.append((xoff, w, sneg))
                else:
                    xgs = []
                    for kc in range(kc_n):
                        xk = xpool.tile([128, GROUP], mm_dt, tag=f"xg{kc}")
                        nc.sync.dma_start(
                            out=xk, in_=xT[kc * 128 : (kc + 1) * 128, tok]
                        )
                        xgs.append(xk)
                    psum_s = ps_s_pool.tile([E, GROUP], F32, tag="ps_a")
                    for kc in range(kc_n):
                        nc.tensor.matmul(
                            psum_s, lhsT=w_sb[:, kc, :], rhs=xgs[kc],
                            start=(kc == 0), stop=(kc == kc_n - 1),
                        )
                    sneg = spool.tile([E, GROUP], F32, tag="sneg")
                    nc.scalar.mul(sneg, psum_s, -1.0)
                    snegs = [(0, GROUP, sneg)]

                og = opool.tile([128, n_sub, E], F32, tag="og")

                for xoff, w, sneg in snegs:
                  for s in range(w // 128):
                    si = xoff // 128 + s
                    # token-major negated scores [128 tok, 64 e] (x W_SCALE)
                    psum_t = ps_t_pool.tile([128, E], F32, tag="ps_t")
                    if fp16:
                        nc.tensor.matmul(
                            psum_t,
                            lhsT=sneg[:, s * 128 : (s + 1) * 128],
                            rhs=identn,
                        )
                    else:
                        nc.tensor.transpose(
                            psum_t, sneg[:, s * 128 : (s + 1) * 128], ident
                        )

                    # exp bias: -max(scores) = inv_s * min(psum_t)
                    mn = smpool.tile([128, 1], F32, tag="mn")
                    nc.vector.tensor_reduce(
                        mn, psum_t, axis=mybir.AxisListType.X, op=mybir.AluOpType.min
                    )
                    # u = exp(scores - max); S = sum(u)
                    u = wkpool.tile([128, E], F32, tag="u")
                    S = smpool.tile([128, 1], F32, tag="S")
                    nc.scalar.activation(
                        u,
                        psum_t,
                        mybir.ActivationFunctionType.Exp,
                        bias=mn,
                        scale=-1.0,
                        accum_out=S,
                    )

                    # threshold = 21st smallest score (negated domain: top-8
                    # of -scores are the smallest scores; 2x8 removed, then
                    # rank 17-24 -> index 4 = 21st)
                    y = wkpool.tile([128, E], F32, tag="y")
                    nc.vector.tensor_copy(y, psum_t)
                    r1 = smpool.tile([128, 8], F32, tag="r1")
                    nc.vector.max(r1, y)
                    nc.vector.match_replace(y, r1, y, NEG_BIG)
                    r2 = smpool.tile([128, 8], F32, tag="r2")
                    nc.vector.max(r2, y)
                    nc.vector.match_replace(y, r2, y, NEG_BIG)
                    r3 = smpool.tile([128, 8], F32, tag="r3")
                    nc.vector.max(r3, y)
                    thr = r3[:, (N_BOTTOM - 16) : (N_BOTTOM - 16 + 1)]

                    # wm = u * (-scores <= thr); ws = sum(wm)
                    wm = wkpool.tile([128, E], F32, tag="wm")
                    ws = smpool.tile([128, 1], F32, tag="ws")
                    nc.vector.scalar_tensor_tensor(
                        out=wm,
                        in0=psum_t,
                        scalar=thr,
                        in1=u,
                        op0=mybir.AluOpType.is_le,
                        op1=mybir.AluOpType.mult,
                        accum_out=ws,
                    )
                    # den = S*eps + ws; out = wm * (1/den)
                    den = smpool.tile([128, 1], F32, tag="den")
                    nc.vector.scalar_tensor_tensor(
                        out=den,
                        in0=S,
                        scalar=EPS,
                        in1=ws,
                        op0=mybir.AluOpType.mult,
                        op1=mybir.AluOpType.add,
                    )
                    rd = smpool.tile([128, 1], F32, tag="rd")
                    nc.vector.reciprocal(rd, den)
                    nc.vector.tensor_scalar_mul(og[:, si, :], wm, rd)

                og_tiles.append(og)

            # all output DMAs at the very end of the SP stream so no x
            # prefetch dispatch ever queues behind an output wait
            for g, og in enumerate(og_tiles):
                nc.sync.dma_start(
                    out=out[g * GROUP : (g + 1) * GROUP, :].rearrange(
                        "(s p) e -> p s e", p=128
                    ),
                    in_=og,
                )

    nc.finalize()
    return nc


_NC = None
LAST_EXEC_NS = None
LAST_RESULTS = None


def _get_nc():
    global _NC
    if _NC is None:
        _NC = _build_nc()
    return _NC


def _split_fp16(a, scale_hi=1.0):
    """a (f32) -> (hi fp16, lo fp16) with a*scale_hi ~= hi + lo/LO_SCALE."""
    s = (a.astype(np.float32) * np.float32(scale_hi)).astype(np.float32)
    hi = s.astype(np.float16)
    lo = ((s - hi.astype(np.float32)) * np.float32(LO_SCALE)).astype(np.float16)
    return hi, lo


def kernel(x, gate_w, trace=False):
    global LAST_EXEC_NS, LAST_RESULTS
    from concourse.bass_utils import run_bass_kernel_spmd

    x = np.asarray(x)
    gate_w = np.asarray(gate_w)
    Btot = x.shape[0] * x.shape[1]
    x2 = np.ascontiguousarray(x.reshape(Btot, D).astype(np.float32, copy=False))
    wt = np.ascontiguousarray(
        gate_w.astype(np.float32, copy=False).T / np.float32(TEMPERATURE)
    )

    nc = _get_nc()
    in_maps = []
    if _MODE == "fp16x3":
        wh, wl = _split_fp16(wt, W_SCALE)
        whl = np.ascontiguousarray(np.concatenate([wh, wl], axis=1))
        ng = TPC // GROUP
        for i in range(N_CORES):
            shard = np.ascontiguousarray(x2[i * TPC : (i + 1) * TPC].T)
            xh, xl = _split_fp16(shard)
            xpk = np.empty((D, ng, 2, GROUP), np.float16)
            xpk[:, :, 0, :] = xh.reshape(D, ng, GROUP)
            xpk[:, :, 1, :] = xl.reshape(D, ng, GROUP)
            in_maps.append({"xpk": xpk.reshape(D, ng * 2 * GROUP), "whl": whl})
    else:
        for i in range(N_CORES):
            shard = np.ascontiguousarray(x2[i * TPC : (i + 1) * TPC].T)
            in_maps.append({"xT": shard, "wT": wt})

    kwargs = {}
    if trace:
        try:
            import antenv.axon_hooks  # noqa: F401  (shimmed by test harness)

            kwargs["trace"] = True
        except ImportError:
            pass
    res = run_bass_kernel_spmd(nc, in_maps, core_ids=list(range(N_CORES)), **kwargs)
    LAST_EXEC_NS = res.exec_time_ns
    LAST_RESULTS = res
    out = np.concatenate([res.results[i]["out"] for i in range(N_CORES)], axis=0)
    return out.reshape(x.shape[0], x.shape[1], E)



# revision 4
# speedup vs baseline: 1.0628x; 1.0628x over previous
"""BoltzmannRouter Trainium2 kernel: 8-core data-parallel Bass implementation.

Full inputs: x (4, 4096, 2048) f32, gate_w (64, 2048) f32.
Output: routing weights (4, 4096, 64) f32 (softmax -> top-44 mask -> renorm).

Sharding: 16384 tokens split 2048/core across 8 NeuronCores; gate weight
replicated.

v2 design (DMA-bound, ~8.75MB/core):
  - x ships as fp16 ONLY (no lo part): halves DMA bytes and matmul passes
    vs the fp16x3 baseline. gate_w keeps an fp16 hi+lo split (free accuracy:
    it rides in the same 128-wide stationary). Simulated rel err 4.5e-3 vs
    the 2e-2 gate.
  - x is packed host-side so every DMA descriptor line is 4KB (4 contraction
    chunks x 512 tokens of one 128-row block), keeping all 16 DMA engines
    busy while arriving in token-group order for pipelining.
  - softmax max-subtraction and the +eps term are dropped (|s|<=3 so exp is
    in [0.05, 25]; eps*S/ws < 1e-6 relative).
  - the hi/lo combine + descale + negate + token-transpose all fold into one
    matmul against a constant stacked-diagonal matrix M (input "mconst").
  - GPSIMD cannot touch PSUM, so Scalar copies scores to SBUF and computes
    u = exp(s) and u2 = exp(-s); the top-k chain (Vector max8/match_replace)
    destroys a GpSimd-made copy of u2 while pristine u2 drives the
    threshold compare; mask*u and the final normalize run on GpSimd.
    Vector keeps only max8/match_replace/reciprocal.
  - output DMAs issue from the Scalar HWDGE queue so SP only streams x.
  - token groups of 256/512/512/512/256 shrink the pipeline head and tail.
"""

import os
import sys

sys.path.insert(0, "/opt/trn_rl_repo")

import numpy as np

D = 2048
E = 64
N_BOTTOM = 20  # 64 experts - 44 active
NEG_BIG = -1e30
TEMPERATURE = 2.718281828459045
N_CORES = 8
TPC = 2048  # tokens per core
GROUPS = (256, 512, 512, 512, 256)  # token groups per core (sum = TPC)
KC = 16  # contraction chunks of 128
CPL = 4  # chunks packed per DMA line
JB = KC // CPL  # row-blocks in the packed x layout

W_SCALE = 64.0  # 2^6: lifts gate_w into fp16-normal range
LO_SCALE = 4096.0  # 2^12: scale on the low fp16 split part of gate_w

_GPSIMD_ACCUM = os.environ.get("BOLTZ_GPSIMD_ACCUM", "1") == "1"


def _build_nc():
    import concourse.bacc as bacc
    import concourse.mybir as mybir
    from concourse.tile import TileContext

    F32 = mybir.dt.float32
    F16 = mybir.dt.float16

    lean_tail = os.environ.get("BOLTZ_LEAN_TAIL", "1") == "1"
    if lean_tail:
        # the stock Tile exit emits drain + barrier + sem-clear + barrier
        # (~8us); the kernel preamble already range-clears the semaphores at
        # the start of every execution, so drain + one barrier suffices
        def _lean_drain_and_barrier(self, tick_clock, wait_clock):
            from concourse.tile import ScopedClock

            drain_inst = self.nc.sync.drain()
            wait_clock.add_sem_waits(
                drain_inst.ins, ScopedClock({None: tick_clock.global_clock})
            )
            self.nc.all_engine_barrier()
            popped = self.nc._tile_sem_poison_stack.pop()
            assert popped is self._sem_poison
            self.sems.allocated()

        TileContext._drain_and_barrier = _lean_drain_and_barrier

    nc = bacc.Bacc(None, target_bir_lowering=False)
    # packed x: row (j*128+p), col layout per group g: [c, t] blocks at 4*t0
    xpk_d = nc.declare_dram_parameter("xpk", [JB * 128, CPL * TPC], F16, isOutput=False)
    whl_d = nc.declare_dram_parameter("whl", [D, 2 * E], F16, isOutput=False)
    # M = [[-I/W_SCALE], [-I/(W_SCALE*LO_SCALE)]]: one matmul by M combines
    # hi+lo, descales, negates and transposes the scores to token-major
    m_d = nc.declare_dram_parameter("mconst", [128, E], F32, isOutput=False)
    out_d = nc.declare_dram_parameter("out", [TPC, E], F16, isOutput=True)

    with TileContext(nc) as tc:
        with (
            tc.tile_pool(name="const", bufs=1) as cpool,
            tc.tile_pool(name="xg", bufs=3) as xpool,
            tc.tile_pool(name="ssb", bufs=2) as spool,
            tc.tile_pool(name="og", bufs=2) as opool,
            tc.tile_pool(name="work", bufs=3) as wkpool,
            tc.tile_pool(name="small", bufs=8) as smpool,
            tc.tile_pool(name="ps_s", bufs=2, space="PSUM") as ps_pool,
            tc.tile_pool(name="ps_t", bufs=4, space="PSUM") as pst_pool,
        ):
            m_sb = cpool.tile([128, E], F32)
            nc.sync.dma_start(out=m_sb, in_=m_d[:, :])
            whl_sb = cpool.tile([128, KC, 2 * E], F16)
            nc.sync.dma_start(
                out=whl_sb, in_=whl_d[:, :].rearrange("(kc p) e -> p kc e", p=128)
            )

            # all x DMAs dispatched up-front on the SP queue, in group order
            xpk_r = xpk_d[:, :].rearrange("(j p) x -> p j x", p=128)
            xgs = []
            t0 = 0
            for L in GROUPS:
                xg = xpool.tile([128, JB, CPL, L], F16, tag=f"xg{L}")
                nc.sync.dma_start(out=xg, in_=xpk_r[:, :, CPL * t0 : CPL * (t0 + L)])
                xgs.append((t0, L, xg))
                t0 += L

            for t0, L, xg in xgs:
                n_sub = L // 128
                # scores: ps rows 0:64 = wh.T@xh (x W_SCALE), 64:128 = wl.T@xh
                ps = ps_pool.tile([128, L], F32, tag=f"ps{L}")
                for kc in range(KC):
                    nc.tensor.matmul(
                        ps,
                        lhsT=whl_sb[:, kc, :],
                        rhs=xg[:, kc // CPL, kc % CPL, :],
                        start=(kc == 0),
                        stop=(kc == KC - 1),
                    )
                # PSUM -> SBUF so PE can re-read it as a stationary operand
                ps_sb = spool.tile([128, L], F32, tag=f"ssb{L}")
                nc.scalar.copy(ps_sb, ps)

                og = opool.tile([128, n_sub, E], F16, tag=f"og{n_sub}")
                for s in range(n_sub):
                    # token-major negated scores: psum_t = -scores [128, 64]
                    psum_t = pst_pool.tile([128, E], F32, tag="ps_t")
                    nc.tensor.matmul(
                        psum_t, lhsT=ps_sb[:, s * 128 : (s + 1) * 128], rhs=m_sb
                    )

                    # u = exp(s), u2 = exp(-s)  (no max-subtraction; |s|<=~3)
                    u = wkpool.tile([128, E], F32, tag="u")
                    nc.scalar.activation(
                        u, psum_t, mybir.ActivationFunctionType.Exp, scale=-1.0
                    )
                    u2 = wkpool.tile([128, E], F32, tag="u2")
                    nc.scalar.activation(
                        u2, psum_t, mybir.ActivationFunctionType.Exp, scale=1.0
                    )

                    # destructible copy of u2 for the top-k chain (SBUF-only)
                    y = wkpool.tile([128, E], F32, tag="y")
                    nc.gpsimd.tensor_copy(y, u2)

                    # largest-20 u2 = bottom-20 scores. 2x(max8+replace)
                    # removes 16, then ranks 17-24 -> index 4 = 21st largest
                    # u2 = 21st smallest score = boundary kept expert.
                    r1 = smpool.tile([128, 8], F32, tag="r1")
                    nc.vector.max(r1, y)
                    nc.vector.match_replace(y, r1, y, NEG_BIG)
                    r2 = smpool.tile([128, 8], F32, tag="r2")
                    nc.vector.max(r2, y)
                    nc.vector.match_replace(y, r2, y, NEG_BIG)
                    r3 = smpool.tile([128, 8], F32, tag="r3")
                    nc.vector.max(r3, y)
                    thr2 = r3[:, (N_BOTTOM - 16) : (N_BOTTOM - 16 + 1)]

                    # wm = u * (u2 <= thr2); ws = sum(wm)
                    wm = wkpool.tile([128, E], F32, tag="wm")
                    ws = smpool.tile([128, 1], F32, tag="ws")
                    nc.vector.scalar_tensor_tensor(
                        out=wm,
                        in0=u2,
                        scalar=thr2,
                        in1=u,
                        op0=mybir.AluOpType.is_le,
                        op1=mybir.AluOpType.mult,
                        accum_out=ws,
                    )
                    # og = wm / ws (f16 cast on write); ws -> 1/ws, unused
                    nc.gpsimd.normalize_recip(og[:, s, :], wm, ws)

                # output DMA from the Scalar HWDGE queue right after the
                # group's last og write: streams during the run without ever
                # blocking SP's x dispatches
                nc.scalar.dma_start(
                    out=out_d[t0 : t0 + L, :].rearrange("(s p) e -> p s e", p=128),
                    in_=og,
                )

    nc.finalize()
    return nc


_NC = None
LAST_EXEC_NS = None
LAST_RESULTS = None


def _get_nc():
    global _NC
    if _NC is None:
        _NC = _build_nc()
    return _NC


def _pack_x(shard_t_f16):
    """[D, TPC] f16 -> [JB*128, CPL*TPC] with 4KB-contiguous group lines."""
    x4 = shard_t_f16.reshape(JB, CPL, 128, TPC)  # [j, c, p, t]
    blocks = []
    t0 = 0
    for L in GROUPS:
        blocks.append(
            x4[:, :, :, t0 : t0 + L].transpose(0, 2, 1, 3).reshape(JB, 128, CPL * L)
        )
        t0 += L
    return np.ascontiguousarray(
        np.concatenate(blocks, axis=2).reshape(JB * 128, CPL * TPC)
    )


def _make_mconst():
    m = np.zeros((128, E), np.float32)
    idx = np.arange(E)
    m[idx, idx] = np.float32(-1.0 / W_SCALE)
    m[E + idx, idx] = np.float32(-1.0 / (W_SCALE * LO_SCALE))
    return m


def kernel(x, gate_w, trace=False):
    global LAST_EXEC_NS, LAST_RESULTS
    from concourse.bass_utils import run_bass_kernel_spmd

    x = np.asarray(x)
    gate_w = np.asarray(gate_w)
    Btot = x.shape[0] * x.shape[1]
    x2 = x.reshape(Btot, D).astype(np.float32, copy=False)

    wt = np.ascontiguousarray(
        gate_w.astype(np.float32, copy=False).T * np.float32(W_SCALE / TEMPERATURE)
    )
    wh = wt.astype(np.float16)
    wl = ((wt - wh.astype(np.float32)) * np.float32(LO_SCALE)).astype(np.float16)
    whl = np.ascontiguousarray(np.concatenate([wh, wl], axis=1))
    mconst = _make_mconst()

    nc = _get_nc()
    in_maps = []
    for i in range(N_CORES):
        shard_t = np.ascontiguousarray(
            x2[i * TPC : (i + 1) * TPC].T.astype(np.float16)
        )
        in_maps.append({"xpk": _pack_x(shard_t), "whl": whl, "mconst": mconst})

    kwargs = {}
    if trace:
        try:
            import antenv.axon_hooks  # noqa: F401  (shimmed by test harness)

            kwargs["trace"] = True
        except ImportError:
            pass
    res = run_bass_kernel_spmd(nc, in_maps, core_ids=list(range(N_CORES)), **kwargs)
    LAST_EXEC_NS = res.exec_time_ns
    LAST_RESULTS = res
    out = np.concatenate(
        [res.results[i]["out"].astype(np.float32) for i in range(N_CORES)], axis=0
    )
    return out.reshape(x.shape[0], x.shape[1], E)


# revision 6
# speedup vs baseline: 1.2620x; 1.1874x over previous
"""BoltzmannRouter Trainium2 kernel: 8-core data-parallel Bass implementation.

Full inputs: x (4, 4096, 2048) f32, gate_w (64, 2048) f32.
Output: routing weights (4, 4096, 64) f32 (softmax -> top-44 mask -> renorm).

Sharding: 16384 tokens split 2048/core across 8 NeuronCores; gate weight
replicated.

v3 design (DMA/PE co-bound, ~8.75MB and ~27us PE per core):
  - x ships as fp16 ONLY (no lo part): halves DMA bytes and matmul passes
    vs the fp16x3 baseline. gate_w keeps an fp16 hi+lo split (free accuracy:
    it rides in the same 128-wide stationary). Simulated rel err 4.7e-3 vs
    the 2e-2 gate.
  - x is packed host-side so every DMA descriptor line is 4KB (4 contraction
    chunks x 512 tokens of one 128-row block), keeping all 16 DMA engines
    busy while arriving in token-group order for pipelining.
  - softmax max-subtraction and the +eps term are dropped (|s|<=3 so exp is
    in [0.05, 25]; eps*S/ws < 1e-6 relative).
  - scores copy to SBUF as fp16 (PSUM can't feed PE stationaries); the hi/lo
    combine + negate + token-transpose fold into one fp16 matmul against a
    constant stacked-diagonal matrix M' = [[-I],[-I*2^-12]]; the 1/W_SCALE
    descale folds into the exp activation scale.
  - engine split: Scalar computes u=exp(s) and u2=exp(-s) (GPSIMD cannot
    touch PSUM); GpSimd makes the destructible top-k copy and runs
    normalize_recip; Vector keeps only max8/match_replace + the masked
    multiply-accumulate.
  - emission is software-pipelined: group g's selection chain is emitted
    AFTER group g+1's matmul/copy/transpose phase so the in-order PE and
    Scalar streams never stall behind the (slow) selection of the previous
    group. Output DMAs sit at the very end of the SP stream.
  - token groups of 256/512/512/512/256 shrink the pipeline head and tail.
"""

import os
import sys

sys.path.insert(0, "/opt/trn_rl_repo")

import numpy as np

D = 2048
E = 64
N_BOTTOM = 20  # 64 experts - 44 active
NEG_BIG = -1e30
TEMPERATURE = 2.718281828459045
N_CORES = 8
TPC = 2048  # tokens per core
GROUPS = (256, 512, 512, 512, 256)  # token groups per core (sum = TPC)
KC = 16  # contraction chunks of 128
CPL = 4  # chunks packed per DMA line
JB = KC // CPL  # row-blocks in the packed x layout

W_SCALE = 64.0  # 2^6: lifts gate_w into fp16-normal range
LO_SCALE = 4096.0  # 2^12: scale on the low fp16 split part of gate_w


def _build_nc():
    import concourse.bacc as bacc
    import concourse.mybir as mybir
    from concourse.tile import TileContext

    F32 = mybir.dt.float32
    F16 = mybir.dt.float16

    lean_tail = os.environ.get("BOLTZ_LEAN_TAIL", "1") == "1"
    if lean_tail:
        # the stock Tile exit emits drain + barrier + sem-clear + barrier
        # (~8us); the kernel preamble already range-clears the semaphores at
        # the start of every execution, so drain + one barrier suffices
        def _lean_drain_and_barrier(self, tick_clock, wait_clock):
            from concourse.tile import ScopedClock

            drain_inst = self.nc.sync.drain()
            wait_clock.add_sem_waits(
                drain_inst.ins, ScopedClock({None: tick_clock.global_clock})
            )
            self.nc.all_engine_barrier()
            popped = self.nc._tile_sem_poison_stack.pop()
            assert popped is self._sem_poison
            self.sems.allocated()

        TileContext._drain_and_barrier = _lean_drain_and_barrier

    nc = bacc.Bacc(None, target_bir_lowering=False)
    # packed x: row (j*128+p), col layout per group g: [c, t] blocks at 4*t0
    xpk_d = nc.declare_dram_parameter("xpk", [JB * 128, CPL * TPC], F16, isOutput=False)
    whl_d = nc.declare_dram_parameter("whl", [D, 2 * E], F16, isOutput=False)
    # M' = [[-I], [-I*2^-12]] (f16): one matmul combines hi+lo, negates and
    # transposes scores to token-major; psum_t = -W_SCALE * s
    m_d = nc.declare_dram_parameter("mconst", [128, E], F16, isOutput=False)
    out_d = nc.declare_dram_parameter("out", [TPC, E], F16, isOutput=True)

    with TileContext(nc) as tc:
        with (
            tc.tile_pool(name="const", bufs=1) as cpool,
            tc.tile_pool(name="xg", bufs=3) as xpool,
            tc.tile_pool(name="ssb", bufs=2) as spool,
            tc.tile_pool(name="og", bufs=5) as opool,
            tc.tile_pool(name="work", bufs=6) as wkpool,
            tc.tile_pool(name="small", bufs=10) as smpool,
            tc.tile_pool(name="ps_s", bufs=2, space="PSUM") as ps_pool,
            tc.tile_pool(name="ps_t", bufs=4, space="PSUM") as pst_pool,
        ):
            m_sb = cpool.tile([128, E], F16)
            nc.sync.dma_start(out=m_sb, in_=m_d[:, :])
            whl_sb = cpool.tile([128, KC, 2 * E], F16)
            nc.sync.dma_start(
                out=whl_sb, in_=whl_d[:, :].rearrange("(kc p) e -> p kc e", p=128)
            )

            # all x DMAs dispatched up-front on the SP queue, in group order
            xpk_r = xpk_d[:, :].rearrange("(j p) x -> p j x", p=128)
            xgs = []
            t0 = 0
            for L in GROUPS:
                xg = xpool.tile([128, JB, CPL, L], F16, tag=f"xg{L}")
                nc.sync.dma_start(out=xg, in_=xpk_r[:, :, CPL * t0 : CPL * (t0 + L)])
                xgs.append((t0, L, xg))
                t0 += L

            out_dmas = []

            def emit_matmul_phase(t0, L, xg):
                n_sub = L // 128
                # scores: ps rows 0:64 = wh.T@xh (x W_SCALE), 64:128 = wl.T@xh
                ps = ps_pool.tile([128, L], F32, tag=f"ps{L}")
                for kc in range(KC):
                    nc.tensor.matmul(
                        ps,
                        lhsT=whl_sb[:, kc, :],
                        rhs=xg[:, kc // CPL, kc % CPL, :],
                        start=(kc == 0),
                        stop=(kc == KC - 1),
                    )
                # PSUM -> SBUF (f16) so PE can re-read it as a stationary
                ps_sb = spool.tile([128, L], F16, tag=f"ssb{L}")
                nc.scalar.copy(ps_sb, ps)
                psum_ts = []
                for s in range(n_sub):
                    # token-major scaled negated scores [128, 64]
                    psum_t = pst_pool.tile([128, E], F32, tag="ps_t")
                    nc.tensor.matmul(
                        psum_t, lhsT=ps_sb[:, s * 128 : (s + 1) * 128], rhs=m_sb
                    )
                    psum_ts.append(psum_t)
                return psum_ts

            def emit_selection(t0, L, psum_ts):
                n_sub = L // 128
                og = opool.tile([128, n_sub, E], F16, tag=f"og_{t0}")
                for s in range(n_sub):
                    psum_t = psum_ts[s]
                    # u = exp(s), u2 = exp(-s): 1/W_SCALE descale folded in
                    u = wkpool.tile([128, E], F32, tag="u")
                    nc.scalar.activation(
                        u,
                        psum_t,
                        mybir.ActivationFunctionType.Exp,
                        scale=-1.0 / W_SCALE,
                    )
                    u2 = wkpool.tile([128, E], F32, tag="u2")
                    nc.scalar.activation(
                        u2,
                        psum_t,
                        mybir.ActivationFunctionType.Exp,
                        scale=1.0 / W_SCALE,
                    )

                    # destructible copy of u2 for the top-k chain (SBUF-only)
                    y = wkpool.tile([128, E], F32, tag="y")
                    nc.gpsimd.tensor_copy(y, u2)

                    # largest-20 u2 = bottom-20 scores. 2x(max8+replace)
                    # removes 16, then ranks 17-24 -> index 4 = 21st largest
                    # u2 = 21st smallest score = boundary kept expert.
                    r1 = smpool.tile([128, 8], F32, tag="r1")
                    nc.vector.max(r1, y)
                    nc.vector.match_replace(y, r1, y, NEG_BIG)
                    r2 = smpool.tile([128, 8], F32, tag="r2")
                    nc.vector.max(r2, y)
                    nc.vector.match_replace(y, r2, y, NEG_BIG)
                    r3 = smpool.tile([128, 8], F32, tag="r3")
                    nc.vector.max(r3, y)
                    thr2 = r3[:, (N_BOTTOM - 16) : (N_BOTTOM - 16 + 1)]

                    # wm = u * (u2 <= thr2); ws = sum(wm)
                    wm = wkpool.tile([128, E], F32, tag="wm")
                    ws = smpool.tile([128, 1], F32, tag="ws")
                    nc.vector.scalar_tensor_tensor(
                        out=wm,
                        in0=u2,
                        scalar=thr2,
                        in1=u,
                        op0=mybir.AluOpType.is_le,
                        op1=mybir.AluOpType.mult,
                        accum_out=ws,
                    )
                    # og = wm / ws (f16 cast on write); ws -> 1/ws, unused
                    nc.gpsimd.normalize_recip(og[:, s, :], wm, ws)
                out_dmas.append((t0, L, og))

            # software pipeline: selection(g-1) is emitted after phase(g) so
            # PE/Scalar in-order streams never wait on selection work
            prev = None
            for t0, L, xg in xgs:
                psum_ts = emit_matmul_phase(t0, L, xg)
                if prev is not None:
                    emit_selection(*prev)
                prev = (t0, L, psum_ts)
            emit_selection(*prev)

            # all output DMAs at the very end of the SP stream so no x
            # dispatch ever queues behind an output wait
            for t0, L, og in out_dmas:
                nc.sync.dma_start(
                    out=out_d[t0 : t0 + L, :].rearrange("(s p) e -> p s e", p=128),
                    in_=og,
                )

    nc.finalize()
    return nc


_NC = None
LAST_EXEC_NS = None
LAST_RESULTS = None


def _get_nc():
    global _NC
    if _NC is None:
        _NC = _build_nc()
    return _NC


def _pack_x(shard_t_f16):
    """[D, TPC] f16 -> [JB*128, CPL*TPC] with 4KB-contiguous group lines."""
    x4 = shard_t_f16.reshape(JB, CPL, 128, TPC)  # [j, c, p, t]
    blocks = []
    t0 = 0
    for L in GROUPS:
        blocks.append(
            x4[:, :, :, t0 : t0 + L].transpose(0, 2, 1, 3).reshape(JB, 128, CPL * L)
        )
        t0 += L
    return np.ascontiguousarray(
        np.concatenate(blocks, axis=2).reshape(JB * 128, CPL * TPC)
    )


def _make_mconst():
    m = np.zeros((128, E), np.float16)
    idx = np.arange(E)
    m[idx, idx] = np.float16(-1.0)
    m[E + idx, idx] = np.float16(-1.0 / LO_SCALE)
    return m


def kernel(x, gate_w, trace=False):
    global LAST_EXEC_NS, LAST_RESULTS
    from concourse.bass_utils import run_bass_kernel_spmd

    x = np.asarray(x)
    gate_w = np.asarray(gate_w)
    Btot = x.shape[0] * x.shape[1]
    x2 = x.reshape(Btot, D).astype(np.float32, copy=False)

    wt = np.ascontiguousarray(
        gate_w.astype(np.float32, copy=False).T * np.float32(W_SCALE / TEMPERATURE)
    )
    wh = wt.astype(np.float16)
    wl = ((wt - wh.astype(np.float32)) * np.float32(LO_SCALE)).astype(np.float16)
    whl = np.ascontiguousarray(np.concatenate([wh, wl], axis=1))
    mconst = _make_mconst()

    nc = _get_nc()
    in_maps = []
    for i in range(N_CORES):
        shard_t = np.ascontiguousarray(
            x2[i * TPC : (i + 1) * TPC].T.astype(np.float16)
        )
        in_maps.append({"xpk": _pack_x(shard_t), "whl": whl, "mconst": mconst})

    kwargs = {}
    if trace:
        try:
            import antenv.axon_hooks  # noqa: F401  (shimmed by test harness)

            kwargs["trace"] = True
        except ImportError:
            pass
    res = run_bass_kernel_spmd(nc, in_maps, core_ids=list(range(N_CORES)), **kwargs)
    LAST_EXEC_NS = res.exec_time_ns
    LAST_RESULTS = res
    out = np.concatenate(
        [res.results[i]["out"].astype(np.float32) for i in range(N_CORES)], axis=0
    )
    return out.reshape(x.shape[0], x.shape[1], E)
